# revision 18
# baseline (speedup 1.0000x reference)
# Trainium2 Bass kernel for per-sample channel-attention module (CAM).
#
# Reference math per sample (C=512, N=H*W=4096):
#   X = x.reshape(C, N)
#   phi = Wp X ; theta = Wt X ; g = Wg X
#   attn = softmax_rows(phi @ theta^T)          # [C, C]
#   y = attn @ g                                 # [C, N]
#   Z = (y^T).flatten().reshape(C, N)            # torch permute+view reinterpretation
#   out = gamma * (Wm @ Z) + x
#
# Algebraic restructuring (cuts PE work ~1.8x vs the naive 6-GEMM chain):
#   G = X X^T                  (Gram, [C, C])
#   L = Wp G Wt^T              (attention logits via two small GEMMs)
#   A' = softmax(L) @ Wg       (fold g-projection into attention)
#   y = A' X                   (single big GEMM)
# The torch permute+view reinterpretation is free: y^T blocks are produced
# with a stride-8 column selection of X as the stationary matmul operand, so
# each PSUM tile lands exactly on a contiguous block of Z's SBUF layout.
#
# G is symmetric, so only the upper-triangle blocks are accumulated on the
# PE (row-block widths 512/384/256/256) and the 5 missing lower blocks are
# reconstructed with cheap PE transposes, cutting Gram PE time ~25%.
#
# Precision plan (validated against the reference in fp64 emulation):
#   - xt (the Gram input stream) is fp16: 10 explicit mantissa bits vs
#     fp32r's 11 — emulated end-to-end rel err is unchanged (7.4e-3 vs
#     gate 2e-2) and it halves the dominant 8MB DMA stream + speeds up
#     the per-tile LDWEIGHTS.
#   - T1 / L / softmax / A' run in float32r (11 explicit mantissa bits):
#     the attention logits are extremely sharp, so coarser rounding there
#     perturbs the softmax too much.
#   - The two post-softmax big GEMMs (y = A' X and the Wm mask projection)
#     run in fp8 e4m3 with DoubleRow perf mode (2 rows/cycle). Scales:
#     apT8 = fp8(16*A'), zs8 = fp8(16*Z) (pure cast of the PSUM tile),
#     wm8 = fp8(128*gamma*Wm^T).
#   - The residual + descale is a single fused scalar_tensor_tensor op on
#     the vector engine: out = mask_psum * (1/2048) + x_bf16. This keeps
#     the residual at bf16 precision and removes both the per-tile PE
#     identity-matmul and the scalar descale copy.
#
# DMA plan: every tensor is staged host-side in its exact SBUF image
# layout (partition-major), so each transfer is contiguous per partition
# with >=2KB lines (1KB-line transfers run at roughly half throughput).
# The output is likewise stored in image layout and unshuffled on the
# host. Queues (per-core bandwidth splits across ACTIVE queues, so
# everything runs on exactly two):
#   sync:   xt even chunks | wtheta | x8  | x q0       | store halves cc 0:2
#   gpsimd: xt odd chunks  | wphi | wg | wm8 | x q1-q3 | store halves cc 2:4
# ZS and mask GEMMs interleave column-by-column (mask column q runs one
# step behind ZS column q); zs casts run on scalar, residual adds on
# vector, and every column store is split across both DMA queues so the
# tail drains in parallel.

import os
import numpy as np
import ml_dtypes

import concourse.bass as bass
import concourse.mybir as mybir
import concourse.tile as tile
from concourse import bacc
from concourse.bass_utils import run_bass_kernel_spmd
from concourse.tile import TileContext
from concourse.masks import make_identity

P = 128          # partitions
C = 512          # channels
N = 4096         # spatial (64*64)
CC = C // P      # 4 channel chunks
NT = N // P      # 32 spatial tiles
QF = N // C      # 8 fold factor for the permute+view reinterpretation
NQ = N // CC     # 1024 spatial quarter width
FP32 = mybir.dt.float32
FP32R = mybir.dt.float32r
FP16 = mybir.dt.float16
BF16 = mybir.dt.bfloat16
F8 = mybir.dt.float8e4

AP_S = 16.0      # A' quantize scale (keeps A' entries out of fp8 subnormals)
WM_S = 128.0     # mask-weight quantize scale
RES_S = AP_S * WM_S  # mask PSUM holds RES_S * (gamma * Wm Z)
DR = mybir.MatmulPerfMode.DoubleRow

# start column of each Gram row-block (row 3 padded to 256 wide so the
# fp32-accumulated block (3,2) comes out directly instead of by transpose)
GOFF = [0, P, 2 * P, 2 * P]
# lower blocks (a, b) reconstructed as transposes of upper blocks (b, a)
GTR = [(1, 0), (2, 0), (3, 0), (2, 1), (3, 1)]


def _f32(ap):
    # reinterpret an fp32r tile as plain fp32 (identical bit layout)
    return ap.bitcast(FP32)


def _build_nc():
    nc = bacc.Bacc("TRN2", target_bir_lowering=False, debug=False, num_devices=8)
    # all inputs/outputs are partition-major SBUF images (see DMA plan)
    x_d = nc.dram_tensor("x", [P, CC, CC, NQ], BF16, kind="ExternalInput").ap()
    xt_d = nc.dram_tensor("xt", [P, NT, C], FP16, kind="ExternalInput").ap()
    x8_d = nc.dram_tensor("x8", [P, CC, N], F8, kind="ExternalInput").ap()
    wphiT_d = nc.dram_tensor("w_phi_t", [P, CC, C], FP32R, kind="ExternalInput").ap()
    wthetaT_d = nc.dram_tensor("w_theta_t", [P, CC, C], FP32R, kind="ExternalInput").ap()
    wg_d = nc.dram_tensor("w_g", [P, CC, C], FP32R, kind="ExternalInput").ap()
    wm8_d = nc.dram_tensor("w_mask8", [P, CC, C], F8, kind="ExternalInput").ap()
    # bf16 output image halves store traffic; host upcasts + unshuffles.
    out_d = nc.dram_tensor("out", [P, QF, CC, C], BF16, kind="ExternalOutput").ap()

    with TileContext(nc) as tc:
        _body(tc, x_d, xt_d, x8_d, wphiT_d, wthetaT_d, wg_d, wm8_d, out_d)
    nc.compile()
    return nc


def _body(tc, x_d, xt_d, x8_d, wphiT_d, wthetaT_d, wg_d, wm8_d, out_d):
    nc = tc.nc
    from contextlib import ExitStack

    with ExitStack() as ctx:
        const = ctx.enter_context(tc.tile_pool(name="const", bufs=1))
        xpool = ctx.enter_context(tc.tile_pool(name="xpool", bufs=1))
        x8pool = ctx.enter_context(tc.tile_pool(name="x8pool", bufs=1))
        wpool = ctx.enter_context(tc.tile_pool(name="wpool", bufs=1))
        bigpool = ctx.enter_context(tc.tile_pool(name="bigpool", bufs=1))
        scratch = ctx.enter_context(tc.tile_pool(name="scratch", bufs=2))
        appool = ctx.enter_context(tc.tile_pool(name="appool", bufs=1))
        zspool = ctx.enter_context(tc.tile_pool(name="zspool", bufs=2))
        vecs = ctx.enter_context(tc.tile_pool(name="vecs", bufs=8))
        outp = ctx.enter_context(tc.tile_pool(name="outp", bufs=4))
        ps = ctx.enter_context(tc.tile_pool(name="ps", bufs=4, space="PSUM"))
        psg = ctx.enter_context(tc.tile_pool(name="psg", bufs=4, space="PSUM"))

        # Throwaway matmuls warm the PE p-state while the first xt chunk is
        # in flight; the Gram stream itself finishes the ramp.
        warm_in = const.tile([P, P], FP32)
        nc.vector.memset(warm_in, 1.0)
        # 7 warms bridge the gap until the first xt chunk's DMA-completion
        # semaphore (~10.5us): any PE idle gap here resets the p-state ramp
        # and the first ~5 Gram tiles would run at half clock.
        warm = psg.tile([P, P], FP32, tag="gacc")
        for _ in range(7):
            nc.tensor.matmul(warm, warm_in, warm_in, start=True, stop=True)

        wphiT = wpool.tile([P, CC, C], FP32R)
        wthetaT = wpool.tile([P, CC, C], FP32R)
        wg_sb = wpool.tile([P, CC, C], FP32R)
        wm8_sb = wpool.tile([P, CC, C], F8)
        x_sb = xpool.tile([P, CC, CC, NQ], BF16)
        x8_sb = x8pool.tile([P, CC, N], F8)

        # ---- stream X^T (fp16) in chunks alternating between the sync and
        # gpsimd DMA queues (2x load bandwidth), folding each tile into the
        # upper-triangle Gram accumulators as soon as its chunk lands.
        # XT[p, t, c] = X[c, 128*t + p];  G[a, b] = sum_n X[a, n] X[b, n].
        xt_sb = bigpool.tile([P, NT, C], FP16, tag="big")
        gacc = [
            psg.tile([P, C - GOFF[i]], FP32, tag="gacc", name=f"gacc{i}")
            for i in range(CC)
        ]
        # Ramped chunk sizes: small first chunks start the Gram stream
        # earlier; steady-state 4-tile chunks keep issue overhead low.
        chunks = [1, 1, 1, 1, 2, 2, 4, 4, 4, 4, 4, 4]
        t0c = 0
        for ki, csz in enumerate(chunks):
            eng = nc.sync if ki % 2 == 0 else nc.gpsimd
            eng.dma_start(
                out=xt_sb[:, t0c:t0c + csz, :],
                in_=xt_d[:, t0c:t0c + csz, :],
            )
            for k in range(csz):
                t = t0c + k
                for mc in range(CC):
                    nc.tensor.matmul(
                        gacc[mc],
                        xt_sb[:, t, mc * P:(mc + 1) * P],
                        xt_sb[:, t, GOFF[mc]:],
                        start=(t == 0),
                        stop=(t == NT - 1),
                    )
            t0c += csz
        assert t0c == NT

        # Weights + x8 + bf16 x quarters queue up behind xt (FIFO keeps
        # them off the Gram stream's bandwidth), ordered by first use.
        nc.sync.dma_start(out=wthetaT, in_=wthetaT_d)
        nc.gpsimd.dma_start(out=wphiT, in_=wphiT_d)
        nc.gpsimd.dma_start(out=wg_sb, in_=wg_d)
        nc.gpsimd.dma_start(out=wm8_sb, in_=wm8_d)
        nc.sync.dma_start(out=x8_sb, in_=x8_d)
        for ci in range(CC):
            eng = nc.sync if ci == 0 else nc.gpsimd
            eng.dma_start(out=x_sb[:, ci], in_=x_d[:, ci])

        # identity (for the transposes) + RES_S*I (residual matmul for the
        # final column); emitted after the DMA triggers so they never delay
        # the queues.
        identity = const.tile([P, P], FP32)
        make_identity(nc, identity)
        eyeS = const.tile([P, P], BF16)
        nc.vector.tensor_scalar_mul(eyeS, identity, RES_S)

        # ---- materialize full G in SBUF: direct copies of the upper
        # blocks, then PE transposes for the 5 missing lower blocks.
        # Row 0 is copied first: it unblocks both T1 jc=0 and the
        # transposes while the remaining copies drain.
        g_sb = scratch.tile([P, CC, C], FP32R, tag="s8")
        # row 0 split across two engines: it gates T1 jc=0 + the transposes
        nc.vector.tensor_copy(g_sb[:, 0, 0:2 * P], gacc[0][:, 0:2 * P])
        nc.scalar.activation(
            out=g_sb[:, 0, 2 * P:], in_=gacc[0][:, 2 * P:],
            func=mybir.ActivationFunctionType.Copy, scale=1.0,
        )
        nc.scalar.activation(
            out=g_sb[:, 1, P:], in_=gacc[1],
            func=mybir.ActivationFunctionType.Copy, scale=1.0,
        )
        nc.vector.tensor_copy(g_sb[:, 2, 2 * P:], gacc[2])
        nc.scalar.activation(
            out=g_sb[:, 3, 2 * P:], in_=gacc[3],
            func=mybir.ActivationFunctionType.Copy, scale=1.0,
        )

        # ---- T1 = G @ Wt^T  (stationary blocks transpose to G by
        # symmetry). jc=0 runs first off row 0 alone; the PE then fills
        # the lower G blocks via transposes before continuing with jc>=1.
        t1_sb = scratch.tile([P, CC, C], FP32R, tag="s8")
        tps = [ps.tile([P, C], FP32, tag="ps", name=f"tp{i}") for i in range(CC)]
        for mc in range(CC):
            nc.tensor.matmul(
                tps[mc],
                g_sb[:, 0, mc * P:(mc + 1) * P],
                wthetaT[:, 0, :],
                start=True,
                stop=False,
            )
        pt_g = psg.tile([P, C], FP32, tag="gacc", name="ptg")
        pt_g2 = psg.tile([P, P], FP32, tag="gacc", name="ptg2")
        for i, (a, b) in enumerate(GTR):
            dst = pt_g[:, i * P:(i + 1) * P] if i < 4 else pt_g2
            nc.tensor.transpose(
                dst, _f32(g_sb[:, b, a * P:(a + 1) * P]), identity
            )
        for i, (a, b) in enumerate(GTR):
            src = pt_g[:, i * P:(i + 1) * P] if i < 4 else pt_g2
            if i % 2 == 0:
                nc.vector.tensor_copy(g_sb[:, a, b * P:(b + 1) * P], src)
            else:
                nc.scalar.activation(
                    out=g_sb[:, a, b * P:(b + 1) * P], in_=src,
                    func=mybir.ActivationFunctionType.Copy, scale=1.0,
                )
        for jc in range(1, CC):
            for mc in range(CC):
                nc.tensor.matmul(
                    tps[mc],
                    g_sb[:, jc, mc * P:(mc + 1) * P],
                    wthetaT[:, jc, :],
                    start=False,
                    stop=(jc == CC - 1),
                )
        for mc in range(CC):
            nc.any.tensor_copy(t1_sb[:, mc, :], tps[mc])

        # ---- L = Wp @ T1 ; softmax rows -> attn
        attn_sb = scratch.tile([P, CC, C], FP32R, tag="s8")
        for mc in range(CC):
            lp = ps.tile([P, C], FP32, tag="ps")
            for ic in range(CC):
                nc.tensor.matmul(
                    lp,
                    wphiT[:, ic, mc * P:(mc + 1) * P],
                    t1_sb[:, ic, :],
                    start=(ic == 0),
                    stop=(ic == CC - 1),
                )
            neg_max = vecs.tile([P, 1], FP32)
            nc.vector.tensor_reduce(
                out=neg_max, in_=lp, axis=mybir.AxisListType.X,
                op=mybir.AluOpType.max, negate=True,
            )
            sums = vecs.tile([P, 1], FP32)
            nc.scalar.activation(
                out=attn_sb[:, mc, :], in_=lp,
                func=mybir.ActivationFunctionType.Exp,
                bias=neg_max, scale=1.0, accum_out=sums,
            )
            rinv = vecs.tile([P, 1], FP32)
            nc.vector.reciprocal(rinv, sums)
            nc.vector.tensor_scalar_mul(
                attn_sb[:, mc, :], attn_sb[:, mc, :], rinv
            )

        # ---- attn^T via PE transposes (fp32 mode; copies round to fp32r).
        # Grouped per softmax block: the transposes of attn block mc run as
        # soon as its softmax finishes, overlapping the later softmaxes.
        attnT_sb = scratch.tile([P, CC, C], FP32R, tag="s8")
        pts = [ps.tile([P, C], FP32, tag="ps", name=f"pt{i}") for i in range(CC)]
        for mc in range(CC):
            for dc in range(CC):
                nc.tensor.transpose(
                    pts[dc][:, mc * P:(mc + 1) * P],
                    _f32(attn_sb[:, mc, dc * P:(dc + 1) * P]),
                    identity,
                )
        for dc in range(CC):
            nc.any.tensor_copy(attnT_sb[:, dc, :], pts[dc])

        # ---- A'^T[j, c] = sum_d Wg[d, j] attn[c, d]; quantize to fp8 with
        # scale AP_S straight from PSUM.
        apT8 = appool.tile([P, CC, C], F8)
        for jc in range(CC):
            ap_ps = ps.tile([P, C], FP32, tag="ps")
            for dc in range(CC):
                nc.tensor.matmul(
                    ap_ps,
                    wg_sb[:, dc, jc * P:(jc + 1) * P],
                    attnT_sb[:, dc, :],
                    start=(dc == 0),
                    stop=(dc == CC - 1),
                )
            nc.scalar.activation(
                out=apT8[:, jc, :], in_=ap_ps,
                func=mybir.ActivationFunctionType.Copy, scale=AP_S,
            )

        # ---- y^T blocks straight into Z layout (fp8 DoubleRow), with the
        # mask GEMM + fused residual + store for column q running one step
        # behind ZS column q. Z[i, q*512 + r] = y^T[8*i + q, r]; with
        # n = 1024*ci + 8*m + q the ZS PSUM tile (ci, q) is exactly
        # zs[:, ci, q-block] of the mask GEMM's moving operand.
        x8r = x8_sb.rearrange("p cc (ci m q) -> p cc ci q m", ci=CC, q=QF)
        zs_tiles = {}
        for k in range(QF + 1):
            if k < QF:
                q = k
                zs8 = zspool.tile([P, CC, C], F8, tag="zs", name=f"zs{q}")
                zs_tiles[q] = zs8
                for ci in range(CC):
                    zp = ps.tile([P, C], FP32, tag="ps")
                    nc.tensor.matmul(
                        zp, x8r[:, 0:2, ci, q, :], apT8[:, 0:2, :],
                        start=True, stop=False, perf_mode=DR,
                    )
                    nc.tensor.matmul(
                        zp, x8r[:, 2:4, ci, q, :], apT8[:, 2:4, :],
                        start=False, stop=True, perf_mode=DR,
                    )
                    # zp = AP_S * y^T block; pure cast keeps the scale.
                    # Casts run on scalar (vector is reserved for the fused
                    # residual ops; gpsimd cannot read PSUM) except the last
                    # column, whose casts split scalar/vector so the final
                    # mask GEMM starts ~1.2us earlier.
                    if q == QF - 1 and ci >= 2:
                        nc.vector.tensor_copy(zs8[:, ci, :], zp)
                    else:
                        nc.scalar.activation(
                            out=zs8[:, ci, :], in_=zp,
                            func=mybir.ActivationFunctionType.Copy, scale=1.0,
                        )
            if k >= 1:
                jb = k - 1
                zs8p = zs_tiles.pop(jb)
                ot = outp.tile([P, CC, C], BF16)
                last = jb == QF - 1
                for oc in range(CC):
                    mp = psg.tile([P, C], FP32, tag="gacc")
                    nc.tensor.matmul(
                        mp, wm8_sb[:, 0:2, oc * P:(oc + 1) * P],
                        zs8p[:, 0:2, :],
                        start=True, stop=False, perf_mode=DR,
                    )
                    nc.tensor.matmul(
                        mp, wm8_sb[:, 2:4, oc * P:(oc + 1) * P],
                        zs8p[:, 2:4, :],
                        start=False, stop=not last, perf_mode=DR,
                    )
                    x_in = x_sb[:, jb // 2, oc, (jb % 2) * C:(jb % 2 + 1) * C]
                    if not last:
                        # fused descale + residual: out = mp/RES_S + x
                        # (bf16) on vector (gpsimd cannot read PSUM)
                        nc.vector.scalar_tensor_tensor(
                            out=ot[:, oc, :], in0=mp, scalar=1.0 / RES_S,
                            in1=x_in,
                            op0=mybir.AluOpType.mult, op1=mybir.AluOpType.add,
                        )
                    else:
                        # final column: the PE is idle at the tail, so the
                        # residual goes through a RES_S*I matmul and the
                        # descales split scalar/vector — two serialized
                        # 600ns vector adds come off the critical path.
                        nc.tensor.matmul(
                            mp, eyeS, x_in, start=False, stop=True,
                        )
                        if oc % 2 == 0:
                            nc.scalar.activation(
                                out=ot[:, oc, :], in_=mp,
                                func=mybir.ActivationFunctionType.Copy,
                                scale=1.0 / RES_S,
                            )
                        else:
                            nc.vector.tensor_scalar_mul(
                                ot[:, oc, :], mp, 1.0 / RES_S
                            )
                        # final column: store each quarter as soon as its
                        # descale lands, alternating queues
                        eng = nc.sync if oc % 2 == 0 else nc.gpsimd
                        eng.dma_start(
                            out=out_d[:, jb, oc, :], in_=ot[:, oc, :]
                        )
                if not last:
                    # every column store is split across both queues so
                    # the tail drains in parallel (2KB-line image halves)
                    nc.sync.dma_start(
                        out=out_d[:, jb, 0:2, :], in_=ot[:, 0:2, :]
                    )
                    nc.gpsimd.dma_start(
                        out=out_d[:, jb, 2:4, :], in_=ot[:, 2:4, :]
                    )


_NC_CACHE = {}
LAST_RESULT = None


def get_nc():
    if "nc" not in _NC_CACHE:
        _NC_CACHE["nc"] = _build_nc()
    return _NC_CACHE["nc"]


def _round_fp32r(x):
    """Round fp32 array to the fp32r grid (11 explicit mantissa bits, RNE)."""
    u = np.ascontiguousarray(x, dtype=np.float32).view(np.uint32).astype(np.uint64)
    shift = 23 - 11
    add = (np.uint64(1) << np.uint64(shift - 1)) - np.uint64(1) + (
        (u >> np.uint64(shift)) & np.uint64(1)
    )
    u = (u + add) & np.uint64(~((1 << shift) - 1) & 0xFFFFFFFF)
    return u.astype(np.uint32).view(np.float32)


def _to_fp8(x):
    """Quantize fp32 -> TRN fp8 e4m3 (max 240) via ml_dtypes."""
    x = np.clip(np.asarray(x, dtype=np.float32), -240.0, 240.0)
    return x.astype(ml_dtypes.float8_e4m3)


def _w_img(w):
    """[C, C]-like -> SBUF image [P, CC, C] (partition-major)."""
    return np.ascontiguousarray(w.reshape(CC, P, C).transpose(1, 0, 2))


def make_in_map(xb, w_phi_t, w_theta_t, w_g, wm8):
    """Per-core input dict; xb is one sample [C, H, W]."""
    xr = _round_fp32r(xb.reshape(C, N))
    xt = xr.T.astype(np.float16)  # [N, C]
    return {
        # x image: [p, ci, cc, nq] = x[cc*128+p, ci*1024+nq]
        "x": np.ascontiguousarray(
            xr.astype(ml_dtypes.bfloat16)
            .reshape(CC, P, CC, NQ).transpose(1, 2, 0, 3)
        ),
        # xt image: [p, t, c] = x[c, t*128+p]
        "xt": np.ascontiguousarray(
            xt.reshape(NT, P, C).transpose(1, 0, 2)
        ),
        # x8 image: [p, cc, n] = fp8(x)[cc*128+p, n]
        "x8": np.ascontiguousarray(
            _to_fp8(xr).reshape(CC, P, N).transpose(1, 0, 2)
        ),
        "w_phi_t": w_phi_t,
        "w_theta_t": w_theta_t,
        "w_g": w_g,
        "w_mask8": wm8,
    }


def prep_weights(w_phi, w_theta, w_g, w_mask, gamma):
    w_phi_t = _w_img(_round_fp32r(np.asarray(w_phi, dtype=np.float32).T))
    w_theta_t = _w_img(_round_fp32r(np.asarray(w_theta, dtype=np.float32).T))
    w_g_r = _w_img(_round_fp32r(np.asarray(w_g, dtype=np.float32)))
    gamma64 = float(np.asarray(gamma, dtype=np.float32).reshape(-1)[0])
    wm8 = _w_img(_to_fp8(
        (np.asarray(w_mask, dtype=np.float64).T * gamma64 * WM_S).astype(np.float32)
    ))
    return w_phi_t, w_theta_t, w_g_r, wm8


def kernel(x, w_phi, w_theta, w_g, w_mask, gamma):
    global LAST_RESULT
    x = np.ascontiguousarray(np.asarray(x, dtype=np.float32))
    B, c, h, w = x.shape
    assert (c, h * w) == (C, N), (x.shape,)

    w_phi_t, w_theta_t, w_g_r, wm8 = prep_weights(
        w_phi, w_theta, w_g, w_mask, gamma
    )
    nc = get_nc()
    in_maps = [
        make_in_map(x[b], w_phi_t, w_theta_t, w_g_r, wm8)
        for b in range(B)
    ]
    trace = bool(int(os.environ.get("KERNEL_TRACE", "0")))
    res = run_bass_kernel_spmd(nc, in_maps, list(range(B)), trace=trace)
    LAST_RESULT = res
    out = np.stack([
        # out image [p, jb, cc, c] -> [C, N]
        np.asarray(res.results[b]["out"]).astype(np.float32)
        .transpose(2, 0, 1, 3).reshape(c, h, w)
        for b in range(B)
    ])
    return out
